# revision 7
# baseline (speedup 1.0000x reference)
"""Trainium2 Bass kernel for MatrixFactorizationIF (embedding-lookup style).

Per batch element b with indices (i, j, k):
    out[b] = ALPHA * <pF[i, :64], M[j]>
           + BETA^2 * sum_s <Vs[i,:,s], M[j]> * <Vg[i,:,s], M[k]>

Architecture (driven by HW measurements: SWDGE gather descriptor
generation on the GPSIMD Q7 cores serializes against DVE/ACT SBUF work
via the shared SBUF port pair, so total ~= T_gen + T_DVE + 0.8*T_ACT;
the TensorEngine has its own ports and overlaps fully):
  - Gathers: 4 SWDGE queues, whole-group single gathers for all three
    streams (pF 512B rows, Mj/Mk 128B rows), queue rotating by 3 each
    group so every queue sees one of each stream type per 4-group
    window (12288 indices), in consumption order.  Fewer, larger
    gathers measurably generate/drain faster per index than chunked
    ones (48 instructions/exec vs 96 for 2-way chunking: ~100us).
  - DVE does ONLY the elementwise products (bf16, 2x packed mode).
    The 64-wide dot reductions run on the TensorEngine as 64
    accumulating identity matmuls into fp32 PSUM (fully overlapped;
    f32 accumulation also halves the output error vs a bf16 tree).
  - ACT upcasts the fp8 V block to bf16 (partially serialized vs gen).
  - Pipeline: gathers(it) | products+PE(it-1) | tail(it-2), so the
    in-order DVE never idles waiting for the PE chain.
  - Host prep: pF repacked to 512B rows [bf16 Pi*ALPHA | fp8 V s-major];
    M to bf16 rows; elements bucketed by (core=i-shard, j%4, k%4) - the
    %4 grouping keeps int16 gather indices in range via strided table
    views - and sorted by i inside each bucket so consecutive pF
    descriptors hit nearby HBM pages.
"""

import numpy as np
import ml_dtypes

N_P = 100000
N_M = 100000
R = 64
S = 3
B = 500000
ALPHA = 0.001
BETA = 0.001

N_CORES = 8
P = 128
PF_SHARD = N_P // N_CORES
NG = 16
EQ = 256                    # packed pF row: 256 int16 = 512 B
CH = 2                      # gather chunks per group
NQ = 4


def _round_up(x, m):
    return -(-x // m) * m


def _raw_gather(g, mybir, out_ap, in_ap, idxs_ap, num_idxs, elem_size,
                elem_step, queue_num, reg=None):
    """dma_gather without the elem_size_bytes%256 assert (non-transpose)."""
    from concourse import ap_utils
    from concourse.bass import MemorySpace, exact_div
    assert idxs_ap.dtype == mybir.dt.int16
    assert in_ap.dtype == out_ap.dtype
    assert in_ap.space == MemorySpace.DRAM
    assert ap_utils.ap_is_contiguous(out_ap.ap[1:])
    assert ap_utils.ap_is_contiguous(idxs_ap.ap[1:])
    assert in_ap.ap[-1][1] == out_ap.ap[-1][1] == elem_size
    assert in_ap.ap[0][0] == elem_step
    stride_bytes = elem_step * mybir.dt.size(in_ap.dtype)
    return g.add_instruction(
        mybir.InstDMAGatherAnt(
            name=g.bass.get_next_instruction_name(),
            ins=[*g.lower_ap_dma(in_ap, for_custom_bir_dma=True),
                 g.lower_ap(idxs_ap),
                 g.lower_val_access(reg if reg is not None
                                    else g.to_reg(num_idxs))],
            outs=[g.lower_ap(out_ap)],
            transpose=False,
            num_idxs=num_idxs,
            elem_size=elem_size,
            stride_bytes_256=exact_div(stride_bytes, 256),
            gen_mode=0,
            single_packet=False,
            queue_num=queue_num,
            sbuf_tokens_per_rank=0,
            sbuf_free_dim_per_rank=0,
            sbuf_free_dim_pad_per_rank=0,
            sbuf_byte_offset=0,
        ))


def build_program(C, repeat=1):
    import concourse.bass as bass
    import concourse.bacc as bacc
    import concourse.mybir as mybir
    from concourse.tile import TileContext

    f32 = mybir.dt.float32
    bf16 = mybir.dt.bfloat16
    i16 = mybir.dt.int16
    f8 = mybir.dt.float8e4
    mult = mybir.AluOpType.mult
    add = mybir.AluOpType.add
    AX = mybir.AxisListType.X

    T = C // P
    C16 = C // 16
    CSZ = C // CH

    nc = bacc.Bacc("TRN2", target_bir_lowering=False, num_swdge_queues=NQ)
    pFq = nc.dram_tensor("pFq", [PF_SHARD, EQ], i16, kind="ExternalInput")
    Mb = nc.dram_tensor("Mb", [N_M, R], bf16, kind="ExternalInput")
    idx = nc.dram_tensor("idx", [P, NG * 3 * C16], i16, kind="ExternalInput")
    ident = nc.dram_tensor("ident", [P, P], bf16, kind="ExternalInput")
    out = nc.dram_tensor("out", [NG * C], f32, kind="ExternalOutput")

    with TileContext(nc) as tc:
        with (
            tc.tile_pool(name="idx", bufs=3) as idx_pool,
            tc.tile_pool(name="pf", bufs=3) as pf_pool,
            tc.tile_pool(name="m", bufs=3) as m_pool,
            tc.tile_pool(name="vq", bufs=2) as vq_pool,
            tc.tile_pool(name="prod", bufs=2) as prod_pool,
            tc.tile_pool(name="small", bufs=2) as small_pool,
            tc.tile_pool(name="res", bufs=2) as res_pool,
            tc.tile_pool(name="const", bufs=1) as const_pool,
            tc.psum_pool(name="ps", bufs=3) as psum_pool,
        ):
            cregS = nc.gpsimd.to_reg(CSZ)
            cregC = nc.gpsimd.to_reg(C)

            ident_t = const_pool.tile([P, P], bf16, tag="ident")
            nc.sync.dma_start(out=ident_t[:], in_=ident[:])

            def emit_gathers(it):
                gl = it % NG
                jc, kc = gl >> 2, gl & 3

                idx_g = idx_pool.tile([P, 3 * C16], i16)
                nc.sync.dma_start(
                    out=idx_g[:], in_=idx[:, gl * 3 * C16:(gl + 1) * 3 * C16])
                idx_t = idx_g[:]

                pf_t = pf_pool.tile([P, T * EQ], i16)
                mj_t = m_pool.tile([P, T * R], bf16, tag="mj")
                mk_t = m_pool.tile([P, T * R], bf16, tag="mk")
                pf4 = pf_t[:].rearrange("p (t e) -> p t e", e=EQ)
                mj3 = mj_t[:].rearrange("p (t r) -> p t r", r=R)
                mk3 = mk_t[:].rearrange("p (t r) -> p t r", r=R)

                mjview = Mb[:].rearrange(
                    "(n f) r -> n (f r)", f=4)[:, jc * R:(jc + 1) * R]
                mkview = Mb[:].rearrange(
                    "(n f) r -> n (f r)", f=4)[:, kc * R:(kc + 1) * R]

                # whole-group single gathers for all three streams; the
                # queue rotates by 3 each group so every queue sees one of
                # each stream type per 4-group window (12288 idx), in
                # consumption order.
                qbase = (3 * it) % NQ
                nc.gpsimd.dma_gather(
                    out_ap=pf4[:],
                    in_ap=pFq[:],
                    idxs_ap=idx_t[:, 0:C16],
                    num_idxs=C, num_idxs_reg=cregC,
                    elem_size=EQ,
                    single_packet=False, queue_num=qbase)
                _raw_gather(
                    nc.gpsimd, mybir,
                    out_ap=mj3[:],
                    in_ap=mjview,
                    idxs_ap=idx_t[:, C16:2 * C16],
                    num_idxs=C, elem_size=R, elem_step=4 * R,
                    queue_num=(qbase + 1) % NQ, reg=cregC)
                _raw_gather(
                    nc.gpsimd, mybir,
                    out_ap=mk3[:],
                    in_ap=mkview,
                    idxs_ap=idx_t[:, 2 * C16:3 * C16],
                    num_idxs=C, elem_size=R, elem_step=4 * R,
                    queue_num=(qbase + 2) % NQ, reg=cregC)
                return pf4, mj3, mk3

            def emit_products(it, tiles):
                pf4, mj3, mk3 = tiles

                # ACT: upcast fp8 V -> bf16
                vq_t = vq_pool.tile([P, T * 6 * R], bf16)
                vq4 = vq_t[:].rearrange("p (t s r) -> p t s r", s=2 * S, r=R)
                vq_flat = vq_t[:].rearrange("p (t e) -> p t e", e=6 * R)
                v8 = pf4[:, :, 64:256].bitcast(f8)
                nc.scalar.copy(out=vq_flat, in_=v8)

                # DVE: products only (bf16 2x), contiguous [P, t, 7, 64]
                pw_t = prod_pool.tile([P, T * 7 * R], bf16)
                pw = pw_t[:].rearrange("p (t s r) -> p t s r", s=7, r=R)
                mjb = mj3[:, :, None, :].to_broadcast([P, T, S, R])
                mkb = mk3[:, :, None, :].to_broadcast([P, T, S, R])
                pi = pf4[:, :, 0:64].bitcast(bf16)
                nc.vector.tensor_tensor(
                    out=pw[:, :, 0, :], in0=pi, in1=mj3[:], op=mult)
                nc.vector.tensor_tensor(
                    out=pw[:, :, 1:4, :], in0=vq4[:, :, 0:3, :], in1=mjb,
                    op=mult)
                nc.vector.tensor_tensor(
                    out=pw[:, :, 4:7, :], in0=vq4[:, :, 3:6, :], in1=mkb,
                    op=mult)

                # PE: r-reduction as 64 accumulating identity matmuls
                ps = psum_pool.tile([P, T * 7], f32)
                for r in range(R):
                    nc.tensor.matmul(
                        ps[:], ident_t[:], pw[:, :, :, r],
                        start=(r == 0), stop=(r == R - 1))
                return ps

            def emit_tail(it, ps):
                gl = it % NG
                dots = ps[:].rearrange("p (t s) -> p t s", s=7)

                # ACT: move the a-dots to SBUF (TT may read only one PSUM
                # operand); DVE tail reads the rest from PSUM directly
                aa_t = small_pool.tile([P, T * S], f32, tag="aa")
                aa = aa_t[:].rearrange("p (t s) -> p t s", s=S)
                nc.scalar.copy(out=aa, in_=dots[:, :, 1:4])
                agp_t = small_pool.tile([P, T * S], f32, tag="agp")
                agp = agp_t[:].rearrange("p (t s) -> p t s", s=S)
                nc.vector.tensor_tensor(
                    out=agp, in0=aa, in1=dots[:, :, 4:7],
                    op=mult)
                ags = small_pool.tile([P, T], f32, tag="ags")
                nc.vector.reduce_sum(out=ags[:], in_=agp, axis=AX)
                res = res_pool.tile([P, T], f32)
                nc.vector.scalar_tensor_tensor(
                    out=res[:], in0=ags[:], scalar=BETA * BETA,
                    in1=dots[:, :, 0], op0=mult, op1=add)
                nc.sync.dma_start(
                    out=out[gl * C:(gl + 1) * C].rearrange(
                        "(t p) -> p t", p=P),
                    in_=res[:])

            tiles = {}
            pss = {}
            for it in range(NG * repeat + 2):
                if it < NG * repeat:
                    tiles[it] = emit_gathers(it)
                if it >= 1 and it - 1 < NG * repeat:
                    pss[it - 1] = emit_products(it - 1, tiles.pop(it - 1))
                if it >= 2:
                    emit_tail(it - 2, pss.pop(it - 2))

    nc.compile()
    return nc


_NC_CACHE = {}


def _get_program(C, repeat=1):
    key = (C, repeat)
    if key not in _NC_CACHE:
        _NC_CACHE[key] = build_program(C, repeat)
    return _NC_CACHE[key]


def pack_tables(pF, M):
    pi_b = (pF[:, :R] * ALPHA).astype(ml_dtypes.bfloat16)
    vs = pF[:, R:(1 + S) * R].reshape(N_P, R, S)
    vg = pF[:, (1 + S) * R:].reshape(N_P, R, S)
    v = np.concatenate(
        [vs.transpose(0, 2, 1).reshape(N_P, S * R),
         vg.transpose(0, 2, 1).reshape(N_P, S * R)], axis=1)
    v8 = v.astype(ml_dtypes.float8_e4m3)
    rows = np.concatenate(
        [pi_b.view(np.uint8).reshape(N_P, 2 * R),
         v8.view(np.uint8).reshape(N_P, 2 * S * R)], axis=1)
    pFq = np.ascontiguousarray(rows).view(np.int16)
    Mb = np.ascontiguousarray(M.astype(ml_dtypes.bfloat16))
    return pFq, Mb


def prepare_inputs(pF, M, ijk):
    i = ijk[:, 0].astype(np.int64)
    j = ijk[:, 1].astype(np.int64)
    k = ijk[:, 2].astype(np.int64)

    core = i // PF_SHARD
    gl = (j & 3) * 4 + (k & 3)
    key = core * NG + gl
    order = np.argsort(key * np.int64(N_P) + i, kind="stable")
    counts = np.bincount(key, minlength=N_CORES * NG)
    C = max(128 * CH, _round_up(int(counts.max()), 128 * CH))
    starts = np.zeros(N_CORES * NG, np.int64)
    starts[1:] = np.cumsum(counts)[:-1]
    nb = len(i)
    rank = np.arange(nb) - np.repeat(starts, counts)
    rank_orig = np.empty(nb, np.int64)
    rank_orig[order] = rank
    src_index = core * (NG * C) + gl * C + rank_orig

    i_loc = (i - core * PF_SHARD).astype(np.int16)
    j_loc = (j >> 2).astype(np.int16)
    k_loc = (k >> 2).astype(np.int16)

    C16 = C // 16
    wrapped = np.zeros((N_CORES, NG, 3, 16, C16), np.int16)
    wp = (rank_orig % 16).astype(np.int64)
    ws = (rank_orig // 16).astype(np.int64)
    wrapped[core, gl, 0, wp, ws] = i_loc
    wrapped[core, gl, 1, wp, ws] = j_loc
    wrapped[core, gl, 2, wp, ws] = k_loc
    wrapped = np.tile(wrapped, (1, 1, 1, 8, 1))
    wrapped = wrapped.reshape(N_CORES, NG * 3, 8, 16, C16).transpose(
        0, 2, 3, 1, 4).reshape(N_CORES, P, NG * 3 * C16)

    pFq, Mb = pack_tables(pF, M)
    ident = np.eye(P, dtype=ml_dtypes.bfloat16)
    in_maps = []
    for c in range(N_CORES):
        in_maps.append({
            "pFq": np.ascontiguousarray(pFq[c * PF_SHARD:(c + 1) * PF_SHARD]),
            "Mb": Mb,
            "idx": np.ascontiguousarray(wrapped[c]),
            "ident": ident,
        })
    return in_maps, src_index, C


def kernel(pF, M, ijk):
    from concourse.bass_utils import run_bass_kernel_spmd

    pF = np.ascontiguousarray(np.asarray(pF, dtype=np.float32))
    M = np.ascontiguousarray(np.asarray(M, dtype=np.float32))
    ijk = np.asarray(ijk)

    in_maps, src_index, C = prepare_inputs(pF, M, ijk)
    nc = _get_program(C)

    results = run_bass_kernel_spmd(
        nc, in_maps, core_ids=list(range(N_CORES))).results

    flat = np.concatenate([results[c]["out"] for c in range(N_CORES)])
    return flat[src_index].astype(np.float32)
